# revision 11
# baseline (speedup 1.0000x reference)
"""Trainium2 Bass kernel for nn_CrossAttention_ResTransformer.

Sharding: token dim n=4096 sharded 8 ways (512 queries/core). K/V are needed
in full on every core (attention is all-to-all over n), so each core computes
the full K/V projections itself (replicated); small weights replicated.

Per-core layout strategy (avoids all on-device transposes):
  - activations are fed to the device pre-transposed (feature-major X^T) and
    augmented with a ones-row so projection biases fold into the matmul
  - scores are computed transposed S^T[j, q] = K @ Q^T, softmax denominator via
    a ones-vector matmul (partition reduction on the PE), attention output
    accumulated as O^T[d, q] = V^T @ exp(S^T)
  - out-projection is done twice: natural a[q, d] (for the feat output +
    residual with fp32 t) and transposed a^T[d, q] (for LayerNorm stats +
    classifier head, both folded into ones/weight matmuls)
  - softmax skips max-subtraction (logits are O(20), exp fits fp32 easily)
"""

import sys

if "/opt/trn_rl_repo" not in sys.path:
    sys.path.insert(0, "/opt/trn_rl_repo")

import numpy as np
import ml_dtypes

import concourse.bass as bass  # noqa: F401  (registers AP types)
import concourse.mybir as mybir
import concourse.tile as tile
from concourse import bacc
from concourse.bass_utils import run_bass_kernel_spmd

BF16 = mybir.dt.bfloat16
F32 = mybir.dt.float32
NPBF16 = ml_dtypes.bfloat16

N = 4096
DIM = 514
OUT = 256
NCLS = 2
EPS = 1e-5
NCORES = 8
NLOC = N // NCORES  # 512
P = 128
DIMA = DIM + 1  # 515: DIM rows + ones row (bias fold)
CH = [(0, 128), (128, 128), (256, 128), (384, 128), (512, 3)]  # chunks of 515
CHO = [(0, 128), (128, 128), (256, 128), (384, 128), (512, 1)]  # chunks of 513
DB5 = [(0, 128), (128, 128), (256, 128), (384, 128), (512, 2)]  # chunks of 514
NT = N // P  # 32 key chunks
# six cross-attention instances, order matches reference stack:
# g: 0=out12,(q2,k1) 1=out13,(q3,k1) 2=out21,(q1,k2) 3=out23,(q3,k2)
#    4=out31,(q1,k3) 5=out32,(q2,k3)
INST = [(1, 0), (2, 0), (0, 1), (2, 1), (0, 2), (1, 2)]
# residual stream sources: mod m -> (g for first OUT block, g for second)
# t1r=[out21,out31] t2r=[out12,out32] t3r=[out13,out23]
RSRC = {0: (2, 4), 1: (0, 5), 2: (1, 3)}

EXP = mybir.ActivationFunctionType.Exp
SQRT = mybir.ActivationFunctionType.Sqrt

_CACHE = {}


def _emit(nc, tc, xt, xq, tr, wq, wo, whg, hc, feat, outT):
    from contextlib import ExitStack

    with ExitStack() as ctx:
        sing = ctx.enter_context(tc.tile_pool(name="sing", bufs=1))

        w_qkv = []
        for m in range(3):
            t = sing.tile([P, 5, 3 * OUT], BF16, name=f"wqkv_s{m}")
            for ci, (r0, rn) in enumerate(CH):
                nc.sync.dma_start(out=t[0:rn, ci, :], in_=wq[m][r0 : r0 + rn, :])
            w_qkv.append(t)
        w_o = []
        for m in range(3):
            t = sing.tile([P, 5, DIM], BF16, name=f"wo_s{m}")
            for ci, (r0, rn) in enumerate(CHO):
                nc.sync.dma_start(out=t[0:rn, ci, :], in_=wo[m][r0 : r0 + rn, :])
            w_o.append(t)
        whg_s = sing.tile([P, 15, 3], BF16, name="whg_s")
        for k in range(15):
            nc.sync.dma_start(out=whg_s[:, k, :], in_=whg[k * 128 : (k + 1) * 128, :])
        hc_s = sing.tile([1, 4], F32, name="hc_s")
        nc.sync.dma_start(out=hc_s[:], in_=hc[:])
        xq_s = []
        for m in range(3):
            t = sing.tile([P, 5, NLOC], BF16, name=f"xq_s{m}")
            for ci, (r0, rn) in enumerate(CH):
                nc.sync.dma_start(out=t[0:rn, ci, :], in_=xq[m][r0 : r0 + rn, :])
            xq_s.append(t)
        ones_col = sing.tile([P, 1], BF16, name="ones_col")
        nc.vector.memset(ones_col[:], 1.0)
        ones_row = sing.tile([1, NLOC], BF16, name="ones_row")
        nc.vector.memset(ones_row[:], 1.0)

        qt_s = [sing.tile([P, 2, NLOC], BF16, name=f"qt_s{m}") for m in range(3)]
        kql_s = [sing.tile([P, 2, NLOC], BF16, name=f"kql_s{m}") for m in range(3)]
        oT_s = sing.tile([P, 6, 2, NLOC], BF16, name="oT_s")
        rT_s = [sing.tile([P, 4, NLOC], BF16, name=f"rT_s{m}") for m in range(3)]
        kt_s = sing.tile([P, 2, N], BF16, name="kt_s")  # K^T of current kv-mod
        v_s = sing.tile([P, NT, OUT], BF16, name="v_s")  # V of current kv-mod

        with (
            tc.tile_pool(name="psA", bufs=1, space="PSUM") as psA,
            tc.tile_pool(name="xtp", bufs=1) as xtp,
            tc.tile_pool(name="atp", bufs=1) as atp,
            tc.tile_pool(name="sml", bufs=1) as sml,
        ):
            # ---- phase A: local Q^T and K_loc^T (bias folded via ones row)
            for m in range(3):
                for dblk in range(2):
                    pq = psA.tile([P, NLOC], F32, tag="pp", bufs=2, name="pq")
                    for ci, (r0, rn) in enumerate(CH):
                        nc.tensor.matmul(
                            pq[:],
                            w_qkv[m][0:rn, ci, dblk * P : (dblk + 1) * P],
                            xq_s[m][0:rn, ci, :],
                            start=(ci == 0),
                            stop=(ci == 4),
                        )
                    nc.scalar.copy(out=qt_s[m][:, dblk, :], in_=pq[:])
                    pk = psA.tile([P, NLOC], F32, tag="pp", bufs=2, name="pk")
                    for ci, (r0, rn) in enumerate(CH):
                        nc.tensor.matmul(
                            pk[:],
                            w_qkv[m][0:rn, ci, OUT + dblk * P : OUT + (dblk + 1) * P],
                            xq_s[m][0:rn, ci, :],
                            start=(ci == 0),
                            stop=(ci == 4),
                        )
                    nc.scalar.copy(out=kql_s[m][:, dblk, :], in_=pk[:])

            # ---- phases B+C interleaved per kv-modality
            for km in range(3):
                # B: full K^T [2P, N] and V [N, OUT] for modality km
                for nt in range(8):
                    xt_t = xtp.tile([P, 5, NLOC], BF16, tag="xt", bufs=3, name="xt_t")
                    for ci, (r0, rn) in enumerate(CH):
                        nc.sync.dma_start(
                            out=xt_t[0:rn, ci, :],
                            in_=xt[km][r0 : r0 + rn, nt * NLOC : (nt + 1) * NLOC],
                        )
                    for dblk in range(2):
                        pk = psA.tile([P, NLOC], F32, tag="pp", bufs=2, name="pkf")
                        for ci, (r0, rn) in enumerate(CH):
                            nc.tensor.matmul(
                                pk[:],
                                w_qkv[km][
                                    0:rn, ci, OUT + dblk * P : OUT + (dblk + 1) * P
                                ],
                                xt_t[0:rn, ci, :],
                                start=(ci == 0),
                                stop=(ci == 4),
                            )
                        nc.scalar.copy(
                            out=kt_s[:, dblk, nt * NLOC : (nt + 1) * NLOC], in_=pk[:]
                        )
                    for sub in range(4):
                        pv = psA.tile([P, NLOC], F32, tag="pp", bufs=2, name="pv")
                        for ci, (r0, rn) in enumerate(CH):
                            nc.tensor.matmul(
                                pv[:, 0:OUT],
                                xt_t[0:rn, ci, sub * P : (sub + 1) * P],
                                w_qkv[km][0:rn, ci, 2 * OUT : 3 * OUT],
                                start=(ci == 0),
                                stop=(ci == 4),
                            )
                        nc.scalar.copy(out=v_s[:, nt * 4 + sub, :], in_=pv[:, 0:OUT])

                # C: the two attention instances reading (K^T, V) of km
                for g in (2 * km, 2 * km + 1):
                    qm = INST[g][0]
                    po = psA.tile([P, 2 * NLOC], F32, tag="po", bufs=2, name="po")
                    pd = psA.tile([1, NLOC], F32, tag="pd", bufs=1, name="pd")
                    for jt in range(NT):
                        pst = psA.tile([P, NLOC], F32, tag="pp", bufs=2, name="pst")
                        for dblk in range(2):
                            nc.tensor.matmul(
                                pst[:],
                                kt_s[:, dblk, jt * P : (jt + 1) * P],
                                qt_s[qm][:, dblk, :],
                                start=(dblk == 0),
                                stop=(dblk == 1),
                            )
                        at_t = atp.tile([P, NLOC], BF16, tag="at", bufs=3, name="at_t")
                        nc.scalar.activation(out=at_t[:], in_=pst[:], func=EXP)
                        for dblk in range(2):
                            nc.tensor.matmul(
                                po[:, dblk * NLOC : (dblk + 1) * NLOC],
                                v_s[:, jt, dblk * P : (dblk + 1) * P],
                                at_t[:],
                                start=(jt == 0),
                                stop=(jt == NT - 1),
                            )
                        nc.tensor.matmul(
                            pd[:],
                            ones_col[:],
                            at_t[:],
                            start=(jt == 0),
                            stop=(jt == NT - 1),
                        )
                    rec = sml.tile([1, NLOC], F32, tag="rec", bufs=2, name="rec")
                    nc.vector.reciprocal(out=rec[:], in_=pd[:])
                    rec_b = sml.tile([P, NLOC], F32, tag="rec_b", bufs=2, name="rec_b")
                    nc.gpsimd.partition_broadcast(rec_b[:], rec[:])
                    for dblk in range(2):
                        nc.vector.tensor_mul(
                            out=oT_s[:, g, dblk, :],
                            in0=po[:, dblk * NLOC : (dblk + 1) * NLOC],
                            in1=rec_b[:],
                        )

        # ---- pass 2+3: residual streams, out-projections, LN stats, head
        with (
            tc.tile_pool(name="psB", bufs=1, space="PSUM") as psB,
            tc.tile_pool(name="wrk", bufs=1) as wrk,
            tc.tile_pool(name="sm2", bufs=1) as sm2,
        ):
            for m in range(3):
                ga, gb = RSRC[m]
                for ci in range(4):
                    g = ga if ci < 2 else gb
                    nc.vector.tensor_add(
                        out=rT_s[m][:, ci, :],
                        in0=oT_s[:, g, ci % 2, :],
                        in1=kql_s[m][:, ci % 2, :],
                    )

            # four single-row accumulators packed into one PSUM bank at
            # 32-aligned partitions via column tiling: P0, P1, colsum, sumsq
            pstat = psB.tile([P, NLOC], F32, tag="pstat", bufs=1, name="pstat")
            ph_r = [pstat[32 * c : 32 * c + 1, :] for c in range(3)]
            pq2 = pstat[96:97, :]
            first = True
            for m in range(3):
                for di, (d0, dn) in enumerate(DB5):
                    pa = psB.tile([P, NLOC], F32, tag="pa", bufs=2, name="pa")
                    for ci, (r0, rn) in enumerate(CHO):
                        rhs = rT_s[m][:, ci, :] if ci < 4 else ones_row[0:1, :]
                        nc.tensor.matmul(
                            pa[0:dn, :],
                            w_o[m][0:rn, ci, d0 : d0 + dn],
                            rhs,
                            start=(ci == 0),
                            stop=(ci == 4),
                        )
                    atT = wrk.tile([P, NLOC], BF16, tag="atT", bufs=3, name="atT")
                    nc.vector.tensor_add(
                        out=atT[0:dn, :], in0=pa[0:dn, :], in1=xq_s[m][0:dn, di, :]
                    )
                    last = m == 2 and di == 4
                    for c in range(3):
                        nc.tensor.matmul(
                            ph_r[c],
                            whg_s[0:dn, m * 5 + di, c : c + 1],
                            atT[0:dn, :],
                            start=first,
                            stop=last,
                            tile_position=(0, 32 * c),
                        )
                    sq = wrk.tile([P, NLOC], BF16, tag="sq", bufs=3, name="sq")
                    nc.vector.tensor_mul(
                        out=sq[0:dn, :], in0=atT[0:dn, :], in1=atT[0:dn, :]
                    )
                    nc.tensor.matmul(
                        pq2,
                        ones_col[0:dn, :],
                        sq[0:dn, :],
                        start=first,
                        stop=last,
                        tile_position=(0, 96),
                    )
                    first = False

            # final logits from transposed stats: [1, NLOC] rows
            inv_d = 1.0 / (3 * DIM)
            mu = sm2.tile([1, NLOC], F32, name="mu")
            nc.vector.tensor_scalar_mul(out=mu[:], in0=ph_r[2], scalar1=inv_d)
            ex2 = sm2.tile([1, NLOC], F32, name="ex2")
            nc.vector.tensor_scalar_mul(out=ex2[:], in0=pq2, scalar1=inv_d)
            mu2 = sm2.tile([1, NLOC], F32, name="mu2")
            nc.vector.tensor_mul(out=mu2[:], in0=mu[:], in1=mu[:])
            var_t = sm2.tile([1, NLOC], F32, name="var_t")
            nc.vector.tensor_sub(out=var_t[:], in0=ex2[:], in1=mu2[:])
            eps_t = sm2.tile([1, 1], F32, name="eps_t")
            nc.vector.memset(eps_t[:], EPS)
            sd = sm2.tile([1, NLOC], F32, name="sd")
            nc.scalar.activation(out=sd[:], in_=var_t[:], func=SQRT, bias=eps_t[:])
            rstd = sm2.tile([1, NLOC], F32, name="rstd")
            nc.vector.reciprocal(out=rstd[:], in_=sd[:])
            for c in range(2):
                tA = sm2.tile([1, NLOC], F32, tag="tA", bufs=2, name="tA")
                nc.vector.tensor_scalar_mul(
                    out=tA[:], in0=mu[:], scalar1=hc_s[0:1, c : c + 1]
                )
                tB = sm2.tile([1, NLOC], F32, tag="tB", bufs=2, name="tB")
                nc.vector.tensor_sub(out=tB[:], in0=ph_r[c], in1=tA[:])
                tC = sm2.tile([1, NLOC], F32, tag="tC", bufs=2, name="tC")
                nc.vector.tensor_mul(out=tC[:], in0=tB[:], in1=rstd[:])
                tD = sm2.tile([1, NLOC], F32, tag="tD", bufs=2, name="tD")
                nc.vector.tensor_scalar_add(
                    out=tD[:], in0=tC[:], scalar1=hc_s[0:1, 2 + c : 3 + c]
                )
                nc.sync.dma_start(out=outT[c : c + 1, :], in_=tD[:])

            # pass 3: natural-layout a (feat output)
            with (
                tc.tile_pool(name="fpool", bufs=1) as fp,
                tc.tile_pool(name="trp", bufs=1) as trp,
            ):
                for qt_i in range(4):
                    ft = fp.tile([P, 3 * DIM], F32, tag="ft", bufs=2, name="ft")
                    for m in range(3):
                        p1 = psB.tile([P, 512], F32, tag="p1", bufs=2, name="p1")
                        p2 = psB.tile([P, 2], F32, tag="p2", bufs=2, name="p2")
                        for ci, (r0, rn) in enumerate(CHO):
                            lhsT = (
                                rT_s[m][:, ci, qt_i * P : (qt_i + 1) * P]
                                if ci < 4
                                else ones_row[0:1, qt_i * P : (qt_i + 1) * P]
                            )
                            nc.tensor.matmul(
                                p1[:],
                                lhsT,
                                w_o[m][0:rn, ci, 0:512],
                                start=(ci == 0),
                                stop=(ci == 4),
                            )
                            nc.tensor.matmul(
                                p2[:],
                                lhsT,
                                w_o[m][0:rn, ci, 512:514],
                                start=(ci == 0),
                                stop=(ci == 4),
                            )
                        tr_t = trp.tile([P, DIM], F32, tag="trt", bufs=2, name="tr_t")
                        nc.sync.dma_start(
                            out=tr_t[:], in_=tr[m][qt_i * P : (qt_i + 1) * P, :]
                        )
                        nc.vector.tensor_add(
                            out=ft[:, m * DIM : m * DIM + 512],
                            in0=p1[:],
                            in1=tr_t[:, 0:512],
                        )
                        nc.vector.tensor_add(
                            out=ft[:, m * DIM + 512 : m * DIM + 514],
                            in0=p2[:],
                            in1=tr_t[:, 512:514],
                        )
                    nc.sync.dma_start(
                        out=feat[qt_i * P : (qt_i + 1) * P, :], in_=ft[:]
                    )


def _build():
    nc = bacc.Bacc("TRN2", target_bir_lowering=False, debug=False, num_devices=NCORES)
    xt = [
        nc.declare_dram_parameter(f"xt{m}", [DIMA, N], BF16, isOutput=False)
        for m in range(3)
    ]
    xq = [
        nc.declare_dram_parameter(f"xq{m}", [DIMA, NLOC], BF16, isOutput=False)
        for m in range(3)
    ]
    tr = [
        nc.declare_dram_parameter(f"tr{m}", [NLOC, DIM], F32, isOutput=False)
        for m in range(3)
    ]
    wq = [
        nc.declare_dram_parameter(f"wqkv{m}", [DIMA, 3 * OUT], BF16, isOutput=False)
        for m in range(3)
    ]
    wo = [
        nc.declare_dram_parameter(f"wo{m}", [513, DIM], BF16, isOutput=False)
        for m in range(3)
    ]
    whg = nc.declare_dram_parameter("whg", [1920, 3], BF16, isOutput=False)
    hc = nc.declare_dram_parameter("hc", [1, 4], F32, isOutput=False)
    feat = nc.declare_dram_parameter("feat", [NLOC, 3 * DIM], F32, isOutput=True)
    outT = nc.declare_dram_parameter("outT", [NCLS, NLOC], F32, isOutput=True)

    with tile.TileContext(nc) as tc:
        _emit(nc, tc, xt, xq, tr, wq, wo, whg, hc, feat, outT)
    nc.finalize()
    return nc


def _get_nc():
    if "nc" not in _CACHE:
        _CACHE["nc"] = _build()
    return _CACHE["nc"]


def _prep_in_maps(inp):
    t = [inp["t1"], inp["t2"], inp["t1c"]]
    Wq = [
        np.concatenate([inp[f"Wqkv{i}"], inp[f"bqkv{i}"][None, :]], 0).astype(NPBF16)
        for i in (1, 2, 3)
    ]
    Wo = [
        np.concatenate([inp[f"Wo{i}"], inp[f"bo{i}"][None, :]], 0).astype(NPBF16)
        for i in (1, 2, 3)
    ]
    g = inp["ln_g"].astype(np.float32)
    b = inp["ln_b"].astype(np.float32)
    Wh = inp["Wh"].astype(np.float32)
    bh = inp["bh"].astype(np.float32)
    gWh = g[:, None] * Wh  # [1542, 2]
    whg = np.zeros((1920, 3), np.float32)
    for m in range(3):
        whg[m * 640 : m * 640 + DIM, 0:2] = gWh[m * DIM : (m + 1) * DIM]
        whg[m * 640 : m * 640 + DIM, 2] = 1.0
    Sg = gWh.sum(0)
    B = b @ Wh + bh
    hc = np.array([[Sg[0], Sg[1], B[0], B[1]]], np.float32)
    whg16 = whg.astype(NPBF16)
    xt_f = [
        np.concatenate([x.T, np.ones((1, N), np.float32)], 0).astype(NPBF16) for x in t
    ]
    in_maps = []
    for c in range(NCORES):
        sl = slice(c * NLOC, (c + 1) * NLOC)
        im = {}
        for m in range(3):
            im[f"xt{m}"] = xt_f[m]
            im[f"xq{m}"] = np.ascontiguousarray(xt_f[m][:, sl])
            im[f"tr{m}"] = np.ascontiguousarray(t[m][sl].astype(np.float32))
            im[f"wqkv{m}"] = Wq[m]
            im[f"wo{m}"] = Wo[m]
        im["whg"] = whg16
        im["hc"] = hc
        in_maps.append(im)
    return in_maps


def kernel(**inputs):
    inp = {k: np.asarray(v) for k, v in inputs.items()}
    nc = _get_nc()
    in_maps = _prep_in_maps(inp)
    res = run_bass_kernel_spmd(nc, in_maps, list(range(NCORES)))
    feat = np.concatenate([res.results[c]["feat"] for c in range(NCORES)], 0)[None]
    out = np.concatenate([res.results[c]["outT"].T for c in range(NCORES)], 0)
    return out.astype(np.float32), feat.astype(np.float32)
